# revision 27
# baseline (speedup 1.0000x reference)
"""CGC multi-gate MoE kernel for Trainium2 (8 NeuronCores, data-parallel over batch).

Problem: 12 experts (4 shared / 4 task0 / 4 task1), each a 2-layer ReLU MLP
D=1024 -> H1=512 -> H2=256, over B=4096 rows; 3 softmax gates combine expert
outputs into t0/t1/ts [B, 256].

Strategy: pure batch data-parallel (512 rows/core, no collectives). Host
pre-transposes x (contraction dim D on SBUF partitions) and packs W1+W2 per
expert into one contiguous [P, 5120] block. Layer-1 output stays transposed
on-chip ([H1, B]); layer-2 output lands with B on partitions for the per-row
gate combines. Outputs leave the device in fp16; the host casts to f32.

Engine assignment (PE ~116us busy is the bottleneck; all others < 80us):
  PE:     L1 (384 mm N=512), L2 (192 mm N=256), gates (96 tiny mm), warmup.
  ACT:    layer-1 relu for m0/m1 (bias=b1); the gate-scaled combine relus
          relu(g*po) = g*relu(po) (softmax g>0) via per-partition scale AP;
          gate exps; ts output-drain dma issue.
  DVE:    layer-1 relu for m2/m3 (m3 chunked per-bt so L2's kh=3 never
          waits); fp16 tensor_tensor adds into fp16 accumulators (2x DVE
          mode); softmax normalize; the last expert's ts tiles come from its
          t1 tiles via a per-row gs/g1 rescale (4x-mode op, no PSUM read).
  Sync:   weight stream (e4/e5 chunked, prefetch distance 2 in the loop so
          drains never block a weight issue) + t0/t1 output drains.
  Scalar queue: x0 (chunked) / b1 / wg / e5-back-half / e6 / xs; x1 issues
          from loop idx1, after the 10-25us DMA crunch.

Ramp design: per-core HBM delivery ramps slowly (~150-250 GB/s until ~25us),
so a ~95-matmul PE warmup (a) holds the HAM clock gate at 8/8 and (b) delays
the first real matmul to ~15us, by which time the two DMA queues are far
enough ahead that the stream runs gap-free (any >2us PE gap drops the clock
to 4/8 for 3-10us). e5 is split across both queues to balance the crunch.
Do NOT issue bulk DMA on the gpsimd SWDGE queue: a third active queue
triggers a sustained ~20% PE utilization throttle.

b2 (and its ones-row bias matmul) is only emitted when b2 != 0 on the host -
the spec fills b2 with zeros; the general path stays available and tested.

Matmul dtype fp16 (default): rel err ~1.1e-3 incl. fp16 accumulate/output.
Also available: "bf16", "f32r".
"""
import os
import sys

for _p in ("/opt/trn_rl_repo", "/root/.axon_site/_ro/trn_rl_repo"):
    if os.path.isdir(_p):
        if _p not in sys.path:
            sys.path.insert(0, _p)
        break

import numpy as np
from contextlib import ExitStack

import ml_dtypes

import concourse.bass as bass
import concourse.mybir as mybir
import concourse.tile as tile
from concourse import bacc
from concourse.bass_utils import run_bass_kernel_spmd

B, D, H1, H2 = 4096, 1024, 512, 256
NE = 12          # 4 shared + 4 task0 + 4 task1
NCORES = 8
BC = B // NCORES # 512 rows per core
P = 128
KO1 = D // P     # 8 contraction chunks, layer 1
KO2 = H1 // P    # 4 contraction chunks, layer 2
MT1 = H1 // P    # 4 output M-tiles, layer 1 (H1 on partitions)
BT = BC // P     # 4 B-tiles per core
GW = 28          # gate logit widths, concatenated: 8 (g0) + 8 (g1) + 12 (gs)
W1LEN = KO1 * H1           # 4096 fp16 words per partition per expert
WPLEN = W1LEN + KO2 * H2   # 5120 incl. packed W2

F32 = mybir.dt.float32
RELU = mybir.ActivationFunctionType.Relu
EXP = mybir.ActivationFunctionType.Exp
MULT = mybir.AluOpType.mult
ADD = mybir.AluOpType.add
MAXOP = mybir.AluOpType.max

# expert processing order: task0 (needs x0, first to arrive), shared, task1
EXPERT_ORDER = [4, 5, 6, 7, 0, 1, 2, 3, 8, 9, 10, 11]
# e4/e5 run before gates exist: plain relu, deferred DVE combine, flushed at
# idx 2 (right after the gs gate is emitted).
DEFER_IDXS = (0, 1)
FLUSH_IDX = 2


def _build_program(dtype, has_b2):
    MMD = {"bf16": mybir.dt.bfloat16, "fp16": mybir.dt.float16,
           "f32r": mybir.dt.float32r}[dtype]
    nc = bacc.Bacc("TRN2", target_bir_lowering=False, debug=False, num_devices=NCORES)

    xst = nc.dram_tensor("xst", [P, KO1, BC], MMD, kind="ExternalInput")
    x0t = nc.dram_tensor("x0t", [P, KO1, BC], MMD, kind="ExternalInput")
    x1t = nc.dram_tensor("x1t", [P, KO1, BC], MMD, kind="ExternalInput")
    wpk = nc.dram_tensor("wpk", [NE, P, WPLEN], MMD, kind="ExternalInput")
    wgd = nc.dram_tensor("wgd", [P, KO1, GW], MMD, kind="ExternalInput")
    b1d = nc.dram_tensor("b1d", [P, NE, MT1], F32, kind="ExternalInput")
    if has_b2:
        b2d = nc.dram_tensor("b2d", [1, NE, H2], MMD, kind="ExternalInput")
        onesd = nc.dram_tensor("onesd", [1, P], MMD, kind="ExternalInput")
    # outputs leave the device in matmul dtype; the host casts to f32
    t0d = nc.dram_tensor("t0d", [P, BT, H2], MMD, kind="ExternalOutput")
    t1d = nc.dram_tensor("t1d", [P, BT, H2], MMD, kind="ExternalOutput")
    tsd = nc.dram_tensor("tsd", [P, BT, H2], MMD, kind="ExternalOutput")

    with tile.TileContext(nc) as tc, ExitStack() as ctx:
        const = ctx.enter_context(tc.tile_pool(name="const", bufs=1))
        xpool = ctx.enter_context(tc.tile_pool(name="xpool", bufs=1))
        wpool = ctx.enter_context(tc.tile_pool(name="wpool", bufs=4))
        hpool = ctx.enter_context(tc.tile_pool(name="hpool", bufs=3))
        opool = ctx.enter_context(tc.tile_pool(name="opool", bufs=9))
        ospool = ctx.enter_context(tc.tile_pool(name="ospool", bufs=12))
        gtmp = ctx.enter_context(tc.tile_pool(name="gtmp", bufs=2))
        l1ps = ctx.enter_context(tc.tile_pool(name="l1ps", bufs=4, space="PSUM"))
        l2ps = ctx.enter_context(tc.tile_pool(name="l2ps", bufs=4, space="PSUM"))

        # ---- warmup constant on the vector engine
        warm_w = const.tile([P, P], MMD, name="warm_w")
        nc.vector.memset(warm_w[:], 1.0)

        # ---- sync queue: weights only. e4 chunked (so matmuls start before
        # the full expert lands; first piece halved to beat cold-queue
        # latency), e5 chunked coarser, e6/e7 whole; later experts issue from
        # the loop at prefetch distance 2 so output drains (also on sync)
        # never sit ahead of a weight issue.
        wtiles = {}
        wtiles[0] = wpool.tile([P, WPLEN], MMD, tag="w", name="w_e4")
        e0_ = EXPERT_ORDER[0]
        for a, b in ((0, 256), (256, 512), (512, 1024), (1024, 2048),
                     (2048, 3072), (3072, 4096), (4096, WPLEN)):
            nc.sync.dma_start(wtiles[0][:, a:b], wpk[e0_, :, a:b])
        # e5 is split across both queues: front half behind e4 on sync,
        # back half behind x0/wg on scalar — each queue carries ~1.8MB
        # through the 10-25us window where HBM delivery is still ramping
        wtiles[1] = wpool.tile([P, WPLEN], MMD, tag="w", name="w_e5")
        e1_ = EXPERT_ORDER[1]
        for a, b in ((0, 1024), (1024, 2048)):
            nc.sync.dma_start(wtiles[1][:, a:b], wpk[e1_, :, a:b])
        # e6 streams on the scalar queue (below); e7 on sync
        wtiles[2] = wpool.tile([P, WPLEN], MMD, tag="w", name="w_i2")
        wtiles[3] = wpool.tile([P, WPLEN], MMD, tag="w", name="w_i3")
        nc.sync.dma_start(wtiles[3][:], wpk[EXPERT_ORDER[3]])

        # ---- scalar queue: x0 chunked, b1 (needed by the first relu), the
        # rest of x0, gate weights, xs, x1 — in first-use order
        x0_sb = xpool.tile([P, KO1, BC], MMD, name="x0_sb")
        xs_sb = xpool.tile([P, KO1, BC], MMD, name="xs_sb")
        x1_sb = xpool.tile([P, KO1, BC], MMD, name="x1_sb")
        b1_sb = const.tile([P, NE, MT1], F32, name="b1_sb")
        wg_sb = const.tile([P, KO1, GW], MMD, name="wg_sb")
        nc.scalar.dma_start(x0_sb[:, 0:1, :], x0t[:, 0:1, :])
        nc.scalar.dma_start(b1_sb[:], b1d[:])
        for ko, n in [(1, 1), (2, 2), (4, 2), (6, 2)]:
            nc.scalar.dma_start(x0_sb[:, ko:ko + n, :], x0t[:, ko:ko + n, :])
        nc.scalar.dma_start(wg_sb[:], wgd[:])
        for a, b in ((2048, 3584), (3584, WPLEN)):
            nc.scalar.dma_start(wtiles[1][:, a:b], wpk[EXPERT_ORDER[1], :, a:b])
        for a, b in ((0, 2048), (2048, WPLEN)):
            nc.scalar.dma_start(wtiles[2][:, a:b], wpk[EXPERT_ORDER[2], :, a:b])
        nc.scalar.dma_start(xs_sb[:], xst[:])
        if has_b2:
            b2_sb = const.tile([1, NE, H2], MMD, name="b2_sb")
            nc.scalar.dma_start(b2_sb[:], b2d[:])
            ones_sb = const.tile([1, P], MMD, name="ones_sb")
            nc.scalar.dma_start(ones_sb[:], onesd[:])

        # ---- HAM warm-up: ~3.4us of dummy PE activity (matches the pstate
        # ramp time) so the clock gate is at 8/8 when real matmuls start
        warm_ps = l2ps.tile([P, H2], F32, tag="l2", name="warm_ps")
        NWARM = 88
        for wi in range(NWARM):
            nc.tensor.matmul(
                warm_ps[:, :P], warm_w[:], warm_w[:],
                start=(wi == 0), stop=(wi == NWARM - 1),
            )
        warm_out = gtmp.tile([P, 1], F32, tag="gn", name="warm_out")
        nc.vector.tensor_reduce(
            warm_out[:], warm_ps[:, :P], axis=mybir.AxisListType.X,
            op=MAXOP,
        )

        g_sb = const.tile([P, BT, GW], F32, name="g_sb")
        rat_sb = const.tile([P, BT], F32, name="rat_sb")  # gs/g1 for e11
        t0a = const.tile([P, BT, H2], MMD, name="t0a")
        t1a = const.tile([P, BT, H2], MMD, name="t1a")
        tsa = const.tile([P, BT, H2], MMD, name="tsa")
        acc_first = {(id(a), bt): True
                     for a in (t0a, t1a, tsa) for bt in range(BT)}

        def emit_gate(src_sb, off, w):
            # logits are ~N(0, 1): |z| < ~7, so exp needs no max-subtraction
            for bt in range(BT):
                psz = l2ps.tile([P, H2], F32, tag="l2", name=f"psz_{off}_{bt}")
                for ko in range(KO1):
                    nc.tensor.matmul(
                        psz[:, :w],
                        src_sb[:, ko, bt * P:(bt + 1) * P],
                        wg_sb[:, ko, off:off + w],
                        start=(ko == 0),
                        stop=(ko == KO1 - 1),
                    )
                e_sb = gtmp.tile([P, GW], F32, tag="ge", name=f"e_sb_{off}_{bt}")
                nc.scalar.activation(e_sb[:, :w], psz[:, :w], EXP)
                ssum = gtmp.tile([P, 1], F32, tag="gs", name=f"ssum_{off}_{bt}")
                nc.vector.tensor_reduce(
                    ssum[:], e_sb[:, :w], axis=mybir.AxisListType.X, op=ADD,
                )
                rsum = gtmp.tile([P, 1], F32, tag="gr", name=f"rsum_{off}_{bt}")
                nc.vector.reciprocal(rsum[:], ssum[:])
                nc.vector.tensor_scalar_mul(
                    g_sb[:, bt, off:off + w], e_sb[:, :w], rsum[:])

        def expert_targets(e):
            if e < 4:
                return [(t0a, 0 + e), (t1a, 8 + e), (tsa, 16 + e)]
            if e < 8:
                return [(t0a, 0 + e), (tsa, 16 + e)]
            return [(t1a, 8 + 4 + (e - 8)), (tsa, 16 + e)]

        def combine_scaled(e, idx, bt, po):
            # relu(g*po) == g*relu(po+?b2-in-po) since softmax g > 0; ACT does
            # the scale for free, DVE only adds (fp16 2x mode). The last
            # expert reuses its t1-scaled relu for ts via a gs/g1 per-row
            # rescale (4x-mode SBUF op) instead of a second PSUM-read relu.
            os_prev = None
            for acc, col in expert_targets(e):
                sc = g_sb[:, bt, col:col + 1]
                last = (idx == 7 and acc is t0a) or idx == 11
                if acc_first[(id(acc), bt)]:
                    acc_first[(id(acc), bt)] = False
                    nc.scalar.activation(acc[:, bt, :], po[:], RELU, scale=sc)
                    continue
                os = ospool.tile([P, H2], MMD, tag="os", name=f"os_{e}_{bt}")
                if idx == 11 and acc is tsa and os_prev is not None:
                    nc.vector.tensor_scalar_mul(
                        os[:], os_prev[:], rat_sb[:, bt:bt + 1])
                else:
                    nc.scalar.activation(os[:], po[:], RELU, scale=sc)
                os_prev = os
                nc.vector.tensor_tensor(
                    acc[:, bt, :], acc[:, bt, :], os[:], op=ADD)
                if last:
                    outd = t0d if acc is t0a else (t1d if acc is t1a else tsd)
                    if acc is tsa:
                        nc.scalar.dma_start(outd[:, bt, :], acc[:, bt, :])
                    else:
                        nc.sync.dma_start(outd[:, bt, :], acc[:, bt, :])

        def combine_deferred(e, bt, o_sb):
            for acc, col in expert_targets(e):
                sc = g_sb[:, bt, col:col + 1]
                if acc_first[(id(acc), bt)]:
                    acc_first[(id(acc), bt)] = False
                    nc.vector.tensor_scalar_mul(acc[:, bt, :], o_sb[:], sc)
                else:
                    nc.vector.scalar_tensor_tensor(
                        acc[:, bt, :], o_sb[:], sc, acc[:, bt, :],
                        op0=MULT, op1=ADD,
                    )

        deferred = {}  # idx -> (e, o_tiles) for DEFER_IDXS, flushed after gs
        for idx, e in enumerate(EXPERT_ORDER):
            src_sb = xs_sb if e < 4 else (x0_sb if e < 8 else x1_sb)

            # x1 is needed first by g1 (idx 4); issuing it from here keeps
            # its 1MB out of the 10-25us DMA crunch
            if idx == 1:
                nc.scalar.dma_start(x1_sb[:], x1t[:])
            # weight prefetch, two experts ahead (e4..e7 issued upfront)
            if 2 <= idx <= 9:
                wtiles[idx + 2] = wpool.tile(
                    [P, WPLEN], MMD, tag="w", name=f"w_i{idx + 2}")
                nc.sync.dma_start(wtiles[idx + 2][:], wpk[EXPERT_ORDER[idx + 2]])
            wt = wtiles[idx]


            # layer 1: hT[H1, BC] = relu(W1[e].T-chunks @ xT + b1[e]), relu on
            # the (otherwise idle) gpsimd engine; last m-tile relu chunked
            # per-bt so layer 2's kh=3 matmuls never wait on it
            hT = hpool.tile([P, MT1, BC], MMD, tag="h", name=f"hT_{e}")
            phs = [l1ps.tile([P, BC], F32, tag="l1", name=f"ph_{e}_{m}")
                   for m in range(MT1)]

            def l1_relu(m):
                # split across ACT (m0/m1, relu-with-bias) and DVE (m2 whole,
                # m3 chunked per-bt so layer 2's kh=3 never waits); gpsimd
                # cannot read PSUM on TRN2
                b1c = b1_sb[:, e, m:m + 1]
                if m < 2:
                    nc.scalar.activation(hT[:, m, :], phs[m][:], RELU, bias=b1c)
                elif m == 2:
                    nc.vector.tensor_scalar(
                        hT[:, m, :], phs[m][:], b1c, 0.0, op0=ADD, op1=MAXOP)
                else:
                    for c in range(BT):
                        nc.vector.tensor_scalar(
                            hT[:, m, c * P:(c + 1) * P],
                            phs[m][:, c * P:(c + 1) * P],
                            b1c, 0.0, op0=ADD, op1=MAXOP)

            if idx < 2:
                # ko-major: each arriving W1/x chunk feeds MT1 matmuls, so the
                # PE keeps pace with the startup DMA stream
                for ko in range(KO1):
                    for m in range(MT1):
                        nc.tensor.matmul(
                            phs[m][:],
                            wt[:, ko * H1 + m * P: ko * H1 + (m + 1) * P],
                            src_sb[:, ko, :],
                            start=(ko == 0),
                            stop=(ko == KO1 - 1),
                        )
                for m in range(MT1):
                    l1_relu(m)
            else:
                for m in range(MT1):
                    for ko in range(KO1):
                        nc.tensor.matmul(
                            phs[m][:],
                            wt[:, ko * H1 + m * P: ko * H1 + (m + 1) * P],
                            src_sb[:, ko, :],
                            start=(ko == 0),
                            stop=(ko == KO1 - 1),
                        )
                    l1_relu(m)

            # gates enter the PE stream after this expert's L1, so a
            # still-streaming x/wg never stalls the L1 chunk pipeline
            if idx == 0:
                emit_gate(x0_sb, 0, 8)    # g0
            elif idx == FLUSH_IDX:
                emit_gate(xs_sb, 16, 12)  # gs (xs resident by now)
            elif idx == 4:
                emit_gate(x1_sb, 8, 8)    # g1
                # per-row gs/g1 ratio for the last expert's tail rescale
                for bt in range(BT):
                    rr = gtmp.tile([P, 1], F32, tag="gr", name=f"rr_{bt}")
                    nc.vector.reciprocal(rr[:], g_sb[:, bt, 15:16])
                    nc.vector.scalar_tensor_tensor(
                        rat_sb[:, bt:bt + 1], g_sb[:, bt, 27:28], 1.0, rr[:],
                        op0=MULT, op1=MULT)

            # layer 2 (+ optional b2 via K=1 ones-row matmul). kh-major
            # keeps all four psum accumulation groups open at once so
            # stop-flag drains overlap other groups' streams (saves ~75ns
            # per group boundary); the last expert stays bt-major so its
            # first output tiles finish early and the tail chains overlap
            # the remaining matmuls.
            o_tiles = []
            pos = [l2ps.tile([P, H2], F32, tag="l2", name=f"po_{e}_{bt}")
                   for bt in range(BT)]
            if idx == 11:
                order = [(kh, bt) for bt in range(BT) for kh in range(KO2)]
            else:
                order = [(kh, bt) for kh in range(KO2) for bt in range(BT)]
            for kh, bt in order:
                nc.tensor.matmul(
                    pos[bt][:],
                    hT[:, kh, bt * P:(bt + 1) * P],
                    wt[:, W1LEN + kh * H2: W1LEN + (kh + 1) * H2],
                    start=(kh == 0),
                    stop=(kh == KO2 - 1 and not has_b2),
                )
            for bt in range(BT):
                po = pos[bt]
                if has_b2:
                    nc.tensor.matmul(
                        po[:], ones_sb[:1, :P], b2_sb[:1, e, :],
                        start=False, stop=True,
                    )
                if idx in DEFER_IDXS:
                    o_sb = opool.tile([P, H2], MMD, tag="o", name=f"o_{e}_{bt}")
                    nc.scalar.activation(o_sb[:], po[:], RELU)
                    o_tiles.append(o_sb)
                else:
                    combine_scaled(e, idx, bt, po)

            if idx in DEFER_IDXS:
                deferred[idx] = (e, o_tiles)
            # deferred combines flush one expert per iteration once gs exists
            if FLUSH_IDX <= idx < FLUSH_IDX + len(DEFER_IDXS):
                de, dtiles = deferred.pop(idx - FLUSH_IDX)
                for bt in range(BT):
                    combine_deferred(de, bt, dtiles[bt])

    nc.finalize()
    return nc


_PROGRAMS = {}


def _get_program(dtype, has_b2):
    key = (dtype, has_b2)
    if key not in _PROGRAMS:
        _PROGRAMS[key] = _build_program(dtype, has_b2)
    return _PROGRAMS[key]


def _prep_inputs(x0, x1, xs, W1, b1, W2, b2, Wg0, Wg1, Wgs, dtype, has_b2):
    """Host-side shard + relayout into the DMA-friendly per-core layouts."""
    f = np.float32
    md = {"bf16": ml_dtypes.bfloat16, "fp16": np.float16, "f32r": np.float32}[dtype]

    def xt_core(x, c):
        # x [B, D] -> core slice transposed/tiled to [P, KO1, BC]
        s = np.asarray(x[c * BC:(c + 1) * BC], f).T          # [D, BC]
        return np.ascontiguousarray(
            s.reshape(KO1, P, BC).transpose(1, 0, 2).astype(md))

    w1r = np.asarray(W1, f).reshape(NE, KO1, P, H1).transpose(0, 2, 1, 3) \
        .reshape(NE, P, W1LEN)
    w2r = np.asarray(W2, f).reshape(NE, KO2, P, H2).transpose(0, 2, 1, 3) \
        .reshape(NE, P, KO2 * H2)
    wpkr = np.ascontiguousarray(
        np.concatenate([w1r, w2r], axis=2).astype(md))
    wgr = np.ascontiguousarray(
        np.concatenate([np.asarray(Wg0, f), np.asarray(Wg1, f), np.asarray(Wgs, f)],
                       axis=1).reshape(KO1, P, GW).transpose(1, 0, 2).astype(md))
    b1r = np.ascontiguousarray(np.asarray(b1, f).reshape(NE, MT1, P).transpose(2, 0, 1))

    base = {"wpk": wpkr, "wgd": wgr, "b1d": b1r}
    if has_b2:
        base["b2d"] = np.ascontiguousarray(np.asarray(b2, f).reshape(1, NE, H2).astype(md))
        base["onesd"] = np.ones((1, P), md)

    in_maps = []
    for c in range(NCORES):
        m = dict(base)
        m["xst"] = xt_core(xs, c)
        m["x0t"] = xt_core(x0, c)
        m["x1t"] = xt_core(x1, c)
        in_maps.append(m)
    return in_maps


def _assemble(results):
    outs = []
    for name in ("t0d", "t1d", "tsd"):
        parts = [
            results[c][name].transpose(1, 0, 2).reshape(BC, H2).astype(np.float32)
            for c in range(NCORES)
        ]
        outs.append(np.ascontiguousarray(np.concatenate(parts, axis=0)))
    return tuple(outs)


def kernel(x0, x1, xs, W1, b1, W2, b2, Wg0, Wg1, Wgs, dtype="fp16", **run_kwargs):
    has_b2 = bool(np.any(np.asarray(b2)))
    nc = _get_program(dtype, has_b2)
    in_maps = _prep_inputs(x0, x1, xs, W1, b1, W2, b2, Wg0, Wg1, Wgs, dtype, has_b2)
    res = run_bass_kernel_spmd(nc, in_maps, core_ids=list(range(NCORES)), **run_kwargs)
    out = _assemble(res.results)
    if run_kwargs:
        return out, res
    return out
